# revision 1
# baseline (speedup 1.0000x reference)
"""Trainium2 Bass kernel for nn_DGLMAG240RGNN (2-layer relational GraphSAGE
+ MLP head), distributed over 8 NeuronCores.

Strategy (graph/data parallel per the sharding hint):
  - dst nodes (and their incident edges) are partitioned contiguously across
    the 8 cores; weights are replicated; the layer-0 output is AllGathered so
    layer 1 can gather arbitrary src rows locally.
  - Per layer, edges are grouped on the host by (dst block of 128, etype),
    padded to tiles of 128 edges (equal tile counts on every core so the SPMD
    program is identical); each tile is gathered from the feature table with
    one batched indirect DMA and segment-mean-reduced with a scaled-one-hot
    matmul on the TensorEngine (PSUM accumulation per block).
  - mean blocks are PE-transposed and spilled to DRAM as meansT [K, D] with
    K = (n_et+1)*in_c; the per-etype projections collapse into one big
    matmul h_pre = meansT.T @ Wstack where Wstack stacks the 5 Wneigh and a
    combined (skip_w + sum_j Wself[j]) block (the self/skip path rides along
    as a pseudo-etype with weight-1 self edges).
  - Linear-layer biases are dropped: they cancel exactly through the
    training-mode batchnorm that follows every linear layer (only mlp_b2
    survives to the output).
  - BatchNorm batch statistics are computed with ones-vector matmuls
    (sum / sum-of-squares) and AllReduced across cores (tiny), then applied
    fused with ELU / ReLU.
Matmuls run as float32r (TF32-like, full PE rate at free dim >= 256).
"""
import math
import os
import sys

sys.path.insert(0, "/opt/trn_rl_repo")

import numpy as np

import concourse.bass as bass
import concourse.mybir as mybir
import concourse.tile as tile
from concourse import bacc

P = 128
F32 = mybir.dt.float32
F32R = mybir.dt.float32r
I32 = mybir.dt.int32
AOT = mybir.AluOpType
ACT = mybir.ActivationFunctionType
EPS = 1e-5
NCORES = 8

FULL_CFG = dict(
    N_SRC0=200000, N_DST0=20000, N_DST1=2048, IN_C=768, HID=1024,
    OUT_C=153, N_ET=5, G=8,
)


# ---------------------------------------------------------------------------
# host-side graph preprocessing
# ---------------------------------------------------------------------------

def prep_layer(src, dst, et, n_dst, n_et, ncores):
    """Partition edges by (core, dst block, etype) into padded 128-edge tiles.

    Returns (groups, per_core, D, NB):
      groups   list of (block, etype, T) shared by all cores; etype == n_et is
               the self-loop pseudo-etype (always T == 1)
      per_core per-core dict of src_t[int32], col_t[f32], w_t[f32], all
               [128, T_total] tile-major
    """
    src = np.ascontiguousarray(src, np.int64)
    dst = np.ascontiguousarray(dst, np.int64)
    et = np.ascontiguousarray(et, np.int64)
    D = n_dst // ncores
    NB = (D + P - 1) // P

    cnt = np.zeros((n_dst, n_et), np.int64)
    np.add.at(cnt, (dst, et), 1)
    w_edge = (1.0 / np.maximum(cnt, 1))[dst, et].astype(np.float32)

    core_of = dst // D
    ldst = dst - core_of * D
    blk = ldst // P
    col = (ldst % P).astype(np.float32)
    key = (core_of * NB + blk) * n_et + et
    order = np.argsort(key, kind="stable")
    so = src[order].astype(np.int32)
    sc = col[order]
    sw = w_edge[order]
    skey = key[order]
    uniq, starts = np.unique(skey, return_index=True)
    starts = np.concatenate([starts, [len(skey)]])
    seg = {int(k): (starts[i], starts[i + 1]) for i, k in enumerate(uniq)}

    # shared tile counts: max over cores per (block, etype)
    groups = []
    for b in range(NB):
        for j in range(n_et):
            mx = 0
            for c in range(ncores):
                k = (c * NB + b) * n_et + j
                if k in seg:
                    s, e = seg[k]
                    mx = max(mx, e - s)
            if mx > 0:
                groups.append((b, j, (mx + P - 1) // P))
        groups.append((b, n_et, 1))

    T_total = sum(g[2] for g in groups)
    per_core = []
    for c in range(ncores):
        src_t = np.zeros((P, T_total), np.int32)
        col_t = np.zeros((P, T_total), np.float32)
        w_t = np.zeros((P, T_total), np.float32)
        t0 = 0
        for (b, j, T) in groups:
            if j == n_et:
                d0 = c * D + b * P
                nv = min(P, D - b * P)
                src_t[:nv, t0] = np.arange(d0, d0 + nv, dtype=np.int32)
                col_t[:nv, t0] = np.arange(nv, dtype=np.float32)
                w_t[:nv, t0] = 1.0
            else:
                k = (c * NB + b) * n_et + j
                if k in seg:
                    s, e = seg[k]
                    n = e - s
                    fs = np.zeros(T * P, np.int32)
                    fc = np.zeros(T * P, np.float32)
                    fw = np.zeros(T * P, np.float32)
                    fs[:n] = so[s:e]
                    fc[:n] = sc[s:e]
                    fw[:n] = sw[s:e]
                    src_t[:, t0:t0 + T] = fs.reshape(T, P).T
                    col_t[:, t0:t0 + T] = fc.reshape(T, P).T
                    w_t[:, t0:t0 + T] = fw.reshape(T, P).T
            t0 += T
        per_core.append(dict(src=src_t, col=col_t, w=w_t))
    return groups, per_core, D, NB


def prep_weights(inp, n_et):
    W0 = np.concatenate(
        [np.asarray(inp["Wneigh0"][j]) for j in range(n_et)]
        + [np.asarray(inp["skip_w0"]) + np.asarray(inp["Wself0"]).sum(0)], axis=0)
    W1 = np.concatenate(
        [np.asarray(inp["Wneigh1"][j]) for j in range(n_et)]
        + [np.asarray(inp["skip_w1"]) + np.asarray(inp["Wself1"]).sum(0)], axis=0)
    return np.ascontiguousarray(W0, np.float32), np.ascontiguousarray(W1, np.float32)


# ---------------------------------------------------------------------------
# device program
# ---------------------------------------------------------------------------

def _splits(n, lim=512):
    """Split n columns into contiguous chunks of <= lim."""
    out = []
    o = 0
    while o < n:
        w = min(lim, n - o)
        out.append((o, w))
        o += w
    return out


def r(ap):
    return ap.bitcast(F32R)


class Prog:
    """Builds the single SPMD bass program."""

    def __init__(self, cfg, groups0, T0, groups1, T1):
        self.cfg = cfg
        c = cfg
        self.K0 = (c["N_ET"] + 1) * c["IN_C"]
        self.K1 = (c["N_ET"] + 1) * c["HID"]
        self.D0 = c["N_DST0"] // NCORES
        self.D1 = c["N_DST1"] // NCORES
        self.NB0 = (self.D0 + P - 1) // P
        self.NB1 = (self.D1 + P - 1) // P
        self.groups0, self.T0 = groups0, T0
        self.groups1, self.T1 = groups1, T1
        self.rg = [list(range(NCORES))]

        nc = bacc.Bacc("TRN2", target_bir_lowering=False, num_devices=NCORES)
        self.nc = nc
        dt_in = lambda nm, sh: nc.dram_tensor(nm, sh, F32, kind="ExternalInput")
        self.x_tab = dt_in("x_tab", [c["N_SRC0"], c["IN_C"]])
        self.src0 = nc.dram_tensor("src0_t", [P, T0], I32, kind="ExternalInput")
        self.col0 = dt_in("col0_t", [P, T0])
        self.w0 = dt_in("w0_t", [P, T0])
        self.src1 = nc.dram_tensor("src1_t", [P, T1], I32, kind="ExternalInput")
        self.col1 = dt_in("col1_t", [P, T1])
        self.w1 = dt_in("w1_t", [P, T1])
        self.Wstack0 = dt_in("Wstack0", [self.K0, c["HID"]])
        self.Wstack1 = dt_in("Wstack1", [self.K1, c["HID"]])
        self.mlp_w1 = dt_in("mlp_w1", [c["HID"], c["HID"]])
        self.OCP = c["OUT_C"] + (c["OUT_C"] % 2)  # fp32r needs even N
        self.mlp_w2 = dt_in("mlp_w2", [c["HID"], self.OCP])
        self.mlp_b2 = dt_in("mlp_b2", [1, c["OUT_C"]])
        self.g0 = dt_in("g0v", [1, c["HID"]])
        self.be0 = dt_in("be0v", [1, c["HID"]])
        self.g1 = dt_in("g1v", [1, c["HID"]])
        self.be1 = dt_in("be1v", [1, c["HID"]])
        self.mlp_g = dt_in("mlp_gv", [1, c["HID"]])
        self.mlp_be = dt_in("mlp_bev", [1, c["HID"]])
        self.out = nc.dram_tensor("out_shard", [self.D1, c["OUT_C"]], F32,
                                  kind="ExternalOutput")
        # internal scratch (exposed as outputs when KDBG=1 for triage)
        import os as _os
        dbg = "ExternalOutput" if _os.environ.get("KDBG") else "Internal"
        self.meansT0 = nc.dram_tensor("meansT0", [self.K0, self.NB0 * P], F32,
                                      kind=dbg)
        self.meansT1 = nc.dram_tensor("meansT1", [self.K1, self.NB1 * P], F32,
                                      kind=dbg)
        self.h0_pre = nc.dram_tensor("h0_pre", [self.NB0 * P, c["HID"]], F32,
                                     kind=dbg)
        self.h0_norm = nc.dram_tensor("h0_norm", [self.D0, c["HID"]], F32)
        self.h0_full = nc.dram_tensor("h0_full", [c["N_DST0"], c["HID"]], F32,
                                      addr_space="Shared")
        self.st_in0 = nc.dram_tensor("st_in0", [1, 2 * c["HID"]], F32)
        self.st_out0 = nc.dram_tensor("st_out0", [1, 2 * c["HID"]], F32,
                                      addr_space="Shared")
        self.st_in1 = nc.dram_tensor("st_in1", [1, 2 * c["HID"]], F32)
        self.st_out1 = nc.dram_tensor("st_out1", [1, 2 * c["HID"]], F32,
                                      addr_space="Shared")
        nch = c["HID"] // P
        self.st_in2 = nc.dram_tensor("st_in2", [P, 2 * nch], F32)
        self.st_out2 = nc.dram_tensor("st_out2", [P, 2 * nch], F32,
                                      addr_space="Shared")

    # -- phase A: gather + one-hot segment means + transpose + spill --------
    def phase_A(self, tc, tag, x_src, srcs_d, cols_d, ws_d, groups, T_total,
                in_c, meansT):
        nc = self.nc
        c = self.cfg
        G = c["G"]
        nf = in_c // P
        half = in_c // 2
        with tc.tile_pool(name=f"{tag}_sg", bufs=1) as sg, \
             tc.tile_pool(name=f"{tag}_msg", bufs=8) as msgp, \
             tc.tile_pool(name=f"{tag}_oh", bufs=4) as ohp, \
             tc.tile_pool(name=f"{tag}_ev", bufs=2) as evp, \
             tc.tile_pool(name=f"{tag}_ps", bufs=2, space="PSUM") as psp, \
             tc.tile_pool(name=f"{tag}_pst", bufs=3, space="PSUM") as pstp:
            srcs = sg.tile([P, T_total], I32, name=f"{tag}_src")
            nc.sync.dma_start(out=srcs[:, :], in_=srcs_d[:, :])
            cols = sg.tile([P, T_total], F32, name=f"{tag}_col")
            nc.sync.dma_start(out=cols[:, :], in_=cols_d[:, :])
            ws = sg.tile([P, T_total], F32, name=f"{tag}_w")
            nc.sync.dma_start(out=ws[:, :], in_=ws_d[:, :])
            iota_i = sg.tile([P, P], I32, name=f"{tag}_ioi")
            nc.gpsimd.iota(iota_i[:, :], pattern=[[1, P]], base=0,
                           channel_multiplier=0)
            iota_f = sg.tile([P, P], F32, name=f"{tag}_iof")
            nc.vector.tensor_copy(out=iota_f[:, :], in_=iota_i[:, :])
            ident = sg.tile([P, P], F32, name=f"{tag}_id")
            from concourse.masks import make_identity
            make_identity(nc, ident[:, :])

            def msg_of(t):
                # HW indirect DMA gathers exactly one row per partition
                m = msgp.tile([P, in_c], F32R, tag="msg",
                              name=f"{tag}_msg")
                nc.gpsimd.indirect_dma_start(
                    out=m[:, :], out_offset=None,
                    in_=x_src[:, :].bitcast(F32R),
                    in_offset=bass.IndirectOffsetOnAxis(
                        ap=srcs[:, t:t + 1], axis=0))
                return m

            t_flat = 0
            for (b, j, T) in groups:
                psA = psp.tile([P, half], F32, tag="psA", name=f"{tag}_psA")
                psB = psp.tile([P, half], F32, tag="psB", name=f"{tag}_psB")
                for k in range(T):
                    t = t_flat + k
                    mg = msg_of(t)[:, :]
                    oh = ohp.tile([P, P], F32R, tag="oh", name=f"{tag}_oh")
                    nc.vector.tensor_scalar(
                        out=oh[:, :], in0=iota_f[:, :],
                        scalar1=cols[:, t:t + 1], scalar2=ws[:, t:t + 1],
                        op0=AOT.is_equal, op1=AOT.mult)
                    st, sp = (k == 0), (k == T - 1)
                    nc.tensor.matmul(psA[:, :], lhsT=oh[:, :],
                                     rhs=mg[:, :half], start=st, stop=sp)
                    nc.tensor.matmul(psB[:, :], lhsT=oh[:, :],
                                     rhs=mg[:, half:], start=st, stop=sp)
                t_flat += T
                ev = evp.tile([P, in_c], F32, tag="ev", name=f"{tag}_ev")
                nc.vector.tensor_copy(out=ev[:, :half], in_=psA[:, :])
                nc.vector.tensor_copy(out=ev[:, half:], in_=psB[:, :])
                stg = evp.tile([P, nf, P], F32, tag="stg", name=f"{tag}_stg")
                for f in range(nf):
                    pst = pstp.tile([P, P], F32, tag="pst", name=f"{tag}_pst")
                    nc.tensor.transpose(pst[:, :], ev[:, f * P:(f + 1) * P],
                                        ident[:, :])
                    nc.vector.tensor_copy(out=stg[:, f, :], in_=pst[:, :])
                dview = meansT[j * in_c:(j + 1) * in_c, b * P:(b + 1) * P]
                dview = dview.rearrange("(f p) d -> p f d", p=P)
                nc.sync.dma_start(out=dview, in_=stg[:, :, :])

    # -- generic BN scale/shift computation from allreduced stats ----------
    def _bn_vec(self, tc, pool, sts, g_d, be_d, n_rows, hid, tag):
        """sts: SBUF [1, 2*hid] (sum | sumsq) -> (scale, shift) [1, hid]."""
        nc = self.nc
        mu = pool.tile([1, hid], F32, name=f"{tag}_mu")
        nc.vector.tensor_scalar(out=mu[:, :], in0=sts[:, :hid],
                                scalar1=1.0 / n_rows, scalar2=None,
                                op0=AOT.mult)
        m2 = pool.tile([1, hid], F32, name=f"{tag}_m2")
        nc.vector.tensor_scalar(out=m2[:, :], in0=sts[:, hid:],
                                scalar1=1.0 / n_rows, scalar2=None,
                                op0=AOT.mult)
        var = pool.tile([1, hid], F32, name=f"{tag}_var")
        nc.vector.tensor_tensor(out=var[:, :], in0=mu[:, :], in1=mu[:, :],
                                op=AOT.mult)
        nc.vector.tensor_tensor(out=var[:, :], in0=m2[:, :], in1=var[:, :],
                                op=AOT.subtract)
        epsa = pool.tile([1, 1], F32, name=f"{tag}_eps")
        nc.vector.memset(epsa[:, :], EPS)
        sd = pool.tile([1, hid], F32, name=f"{tag}_sd")
        nc.scalar.activation(out=sd[:, :], in_=var[:, :], func=ACT.Sqrt,
                             bias=epsa[:, :])
        rstd = pool.tile([1, hid], F32, name=f"{tag}_rstd")
        nc.vector.reciprocal(out=rstd[:, :], in_=sd[:, :])
        gv = pool.tile([1, hid], F32, name=f"{tag}_gv")
        nc.sync.dma_start(out=gv[:, :], in_=g_d[:, :])
        bv = pool.tile([1, hid], F32, name=f"{tag}_bv")
        nc.sync.dma_start(out=bv[:, :], in_=be_d[:, :])
        scale = pool.tile([1, hid], F32R, name=f"{tag}_scale")
        nc.vector.tensor_tensor(out=scale[:, :], in0=gv[:, :], in1=rstd[:, :],
                                op=AOT.mult)
        shift = pool.tile([1, hid], F32R, name=f"{tag}_shift")
        nc.vector.tensor_tensor(out=shift[:, :], in0=mu[:, :],
                                in1=scale[:, :].bitcast(F32), op=AOT.mult)
        nc.vector.tensor_tensor(out=shift[:, :], in0=bv[:, :], in1=shift[:, :],
                                op=AOT.subtract)
        return scale, shift

    def _bn_bcast(self, tc, pool, psp, scale, shift, hid, tag):
        """Broadcast [1, hid] scale/shift to [P, hid] SBUF tiles via PE."""
        nc = self.nc
        ones_f = pool.tile([1, P], F32, name=f"{tag}_onesf")
        nc.vector.memset(ones_f[:, :], 1.0)
        ones_r = pool.tile([1, P], F32R, name=f"{tag}_ones")
        nc.vector.tensor_copy(out=ones_r[:, :], in_=ones_f[:, :])
        scB = pool.tile([P, hid], F32, name=f"{tag}_scB")
        shB = pool.tile([P, hid], F32, name=f"{tag}_shB")
        for (o, w) in _splits(hid):
            pb = psp.tile([P, w], F32, tag="bc", name=f"{tag}_pbc")
            nc.tensor.matmul(pb[:, :], lhsT=ones_r[:, :],
                             rhs=scale[:, o:o + w], start=True, stop=True)
            nc.vector.tensor_copy(out=scB[:, o:o + w], in_=pb[:, :])
            pb2 = psp.tile([P, w], F32, tag="bc", name=f"{tag}_pbc2")
            nc.tensor.matmul(pb2[:, :], lhsT=ones_r[:, :],
                             rhs=shift[:, o:o + w], start=True, stop=True)
            nc.vector.tensor_copy(out=shB[:, o:o + w], in_=pb2[:, :])
        return scB, shB

    def _bn_elu_tile(self, pool, y, nv, scB, shB, tag, relu=False):
        """In-place BN-apply + ELU/ReLU on SBUF tile y [P, hid] (valid nv)."""
        nc = self.nc
        yv = y[:nv, :]
        nc.vector.tensor_tensor(out=yv, in0=yv, in1=scB[:nv, :], op=AOT.mult)
        nc.vector.tensor_tensor(out=yv, in0=yv, in1=shB[:nv, :], op=AOT.add)
        if relu:
            nc.vector.tensor_scalar(out=yv, in0=yv, scalar1=0.0, scalar2=None,
                                    op0=AOT.max)
            return
        hid = y.shape[1]
        mn = pool.tile([P, hid], F32, tag="elu_m", name=f"{tag}_m")
        nc.vector.tensor_scalar(out=mn[:nv, :], in0=yv, scalar1=0.0,
                                scalar2=None, op0=AOT.min)
        ex = pool.tile([P, hid], F32, tag="elu_e", name=f"{tag}_e")
        nc.scalar.activation(out=ex[:nv, :], in_=mn[:nv, :], func=ACT.Exp)
        nc.vector.tensor_scalar(out=ex[:nv, :], in0=ex[:nv, :], scalar1=1.0,
                                scalar2=None, op0=AOT.subtract)
        nc.vector.tensor_tensor(out=yv, in0=yv, in1=ex[:nv, :], op=AOT.max)

    # -- phase B0: big matmul for layer 0 + stats + BN + ELU + AllGather ----
    def phase_B0(self, tc):
        nc = self.nc
        c = self.cfg
        hid = c["HID"]
        nk = self.K0 // P
        hs_splits = _splits(hid)
        with tc.tile_pool(name="b0_w", bufs=1) as wp, \
             tc.tile_pool(name="b0_m", bufs=2) as mp, \
             tc.tile_pool(name="b0_h", bufs=2) as hp, \
             tc.tile_pool(name="b0_misc", bufs=1) as miscp:
            wst = wp.tile([P, nk, hid], F32R, name="b0_wst")
            nc.sync.dma_start(
                out=wst[:, :, :],
                in_=self.Wstack0[:, :].rearrange("(k p) n -> p k n",
                                                 p=P).bitcast(F32R))
            ones_cf = miscp.tile([P, 1], F32, name="b0_onesf")
            nc.vector.memset(ones_cf[:, :], 1.0)
            ones_c = miscp.tile([P, 1], F32R, name="b0_ones")
            nc.vector.tensor_copy(out=ones_c[:, :], in_=ones_cf[:, :])
            with tc.tile_pool(name="b0_pss", bufs=1, space="PSUM") as pss:
                ps_s = [pss.tile([1, w], F32, tag=f"s{i}", name=f"b0_pss{i}")
                        for i, (o, w) in enumerate(hs_splits)]
                ps_q = [pss.tile([1, w], F32, tag=f"q{i}", name=f"b0_psq{i}")
                        for i, (o, w) in enumerate(hs_splits)]
                with tc.tile_pool(name="b0_ph", bufs=2, space="PSUM") as php:
                    for d in range(self.NB0):
                        nv = min(P, self.D0 - d * P)
                        phs = [php.tile([P, w], F32, tag=f"h{i}",
                                        name=f"b0_ph{i}")
                               for i, (o, w) in enumerate(hs_splits)]
                        MKC = 12
                        for k0 in range(0, nk, MKC):
                            kn = min(MKC, nk - k0)
                            mt = mp.tile([P, MKC, P], F32R, tag="mt",
                                         name="b0_mt")
                            nc.sync.dma_start(
                                out=mt[:, :kn, :],
                                in_=self.meansT0[k0 * P:(k0 + kn) * P,
                                                 d * P:(d + 1) * P]
                                .rearrange("(k p) d -> p k d",
                                           p=P).bitcast(F32R))
                            for k in range(kn):
                                for i, (o, w) in enumerate(hs_splits):
                                    nc.tensor.matmul(
                                        phs[i][:, :], lhsT=mt[:, k, :],
                                        rhs=wst[:, k0 + k, o:o + w],
                                        start=(k0 + k == 0),
                                        stop=(k0 + k == nk - 1))
                        hs = hp.tile([P, hid], F32R, tag="hs", name="b0_hs")
                        for i, (o, w) in enumerate(hs_splits):
                            nc.vector.tensor_copy(out=hs[:, o:o + w],
                                               in_=phs[i][:, :])
                        nc.sync.dma_start(
                            out=self.h0_pre[d * P:d * P + nv, :].bitcast(F32R),
                            in_=hs[:nv, :])
                        sq = hp.tile([P, hid], F32R, tag="sq", name="b0_sq")
                        nc.vector.tensor_tensor(out=sq[:nv, :], in0=hs[:nv, :],
                                                in1=hs[:nv, :], op=AOT.mult)
                        st, sp = (d == 0), (d == self.NB0 - 1)
                        for i, (o, w) in enumerate(hs_splits):
                            nc.tensor.matmul(ps_s[i][:, :],
                                             lhsT=ones_c[:nv, :],
                                             rhs=hs[:nv, o:o + w],
                                             start=st, stop=sp)
                            nc.tensor.matmul(ps_q[i][:, :],
                                             lhsT=ones_c[:nv, :],
                                             rhs=sq[:nv, o:o + w],
                                             start=st, stop=sp)
                stv = miscp.tile([1, 2 * hid], F32, name="b0_stv")
                for i, (o, w) in enumerate(hs_splits):
                    nc.vector.tensor_copy(out=stv[:, o:o + w], in_=ps_s[i][:, :])
                    nc.vector.tensor_copy(out=stv[:, hid + o:hid + o + w],
                                       in_=ps_q[i][:, :])
            nc.sync.dma_start(out=self.st_in0[:, :], in_=stv[:, :])
            nc.gpsimd.collective_compute(
                "AllReduce", AOT.add, replica_groups=self.rg,
                ins=[self.st_in0[:, :]], outs=[self.st_out0[:, :]])
        # scope 2: Wstack pool closed; BN vectors + apply + ELU
        with tc.tile_pool(name="b0_sg", bufs=1) as sgp, \
             tc.tile_pool(name="b0_ap", bufs=2) as app:
            sts = sgp.tile([1, 2 * hid], F32, name="b0_sts")
            nc.sync.dma_start(out=sts[:, :], in_=self.st_out0[:, :])
            scale, shift = self._bn_vec(tc, sgp, sts, self.g0, self.be0,
                                        c["N_DST0"], hid, "bn0")
            with tc.tile_pool(name="b0_psb", bufs=2, space="PSUM") as psb:
                scB, shB = self._bn_bcast(tc, sgp, psb, scale, shift, hid,
                                          "bn0")
            # phase C0: apply BN + ELU, write h0_norm
            for d in range(self.NB0):
                nv = min(P, self.D0 - d * P)
                y = app.tile([P, hid], F32, tag="y", name="c0_y")
                nc.sync.dma_start(out=y[:nv, :],
                                  in_=self.h0_pre[d * P:d * P + nv, :])
                self._bn_elu_tile(app, y, nv, scB, shB, f"c0_{d}")
                nc.sync.dma_start(out=self.h0_norm[d * P:d * P + nv, :],
                                  in_=y[:nv, :])
        nc.gpsimd.collective_compute(
            "AllGather", AOT.bypass, replica_groups=self.rg,
            ins=[self.h0_norm[:, :]], outs=[self.h0_full[:, :]])

    # -- phase B1 + BN1 + MLP head -----------------------------------------
    def phase_B1(self, tc):
        nc = self.nc
        c = self.cfg
        hid = c["HID"]
        nk = self.K1 // P
        nd = self.NB1
        nch = hid // P
        hs_splits = _splits(hid)
        with tc.tile_pool(name="b1_h", bufs=1) as hp:
            ones_cf = hp.tile([P, 1], F32, name="b1_onesf")
            nc.vector.memset(ones_cf[:, :], 1.0)
            ones_c = hp.tile([P, 1], F32R, name="b1_ones")
            nc.vector.tensor_copy(out=ones_c[:, :], in_=ones_cf[:, :])
            h1 = [hp.tile([P, hid], F32R, name=f"b1_h1_{d}")
                  for d in range(nd)]
            stv = hp.tile([1, 2 * hid], F32, name="b1_stv")
            with tc.tile_pool(name="b1_pss", bufs=1, space="PSUM") as pss:
                ps_s = [pss.tile([1, w], F32, tag=f"s{i}", name=f"b1_pss{i}")
                        for i, (o, w) in enumerate(hs_splits)]
                ps_q = [pss.tile([1, w], F32, tag=f"q{i}", name=f"b1_psq{i}")
                        for i, (o, w) in enumerate(hs_splits)]
                with tc.tile_pool(name="b1_w", bufs=3) as wp, \
                     tc.tile_pool(name="b1_mt", bufs=1) as mtp, \
                     tc.tile_pool(name="b1_ph", bufs=1, space="PSUM") as php:
                    phs = [[php.tile([P, w], F32, tag=f"h{d}_{i}",
                                     name=f"b1_ph{d}_{i}")
                            for i, (o, w) in enumerate(hs_splits)]
                           for d in range(nd)]
                    mts = []
                    for d in range(nd):
                        mtd = mtp.tile([P, nk, P], F32R, tag=f"mt{d}",
                                       name=f"b1_mt{d}")
                        nc.sync.dma_start(
                            out=mtd[:, :, :],
                            in_=self.meansT1[:, d * P:(d + 1) * P]
                            .rearrange("(k p) d -> p k d", p=P).bitcast(F32R))
                        mts.append(mtd)
                    for k in range(nk):
                        wch = wp.tile([P, hid], F32R, tag="wch",
                                      name="b1_wch")
                        nc.sync.dma_start(
                            out=wch[:, :],
                            in_=self.Wstack1[k * P:(k + 1) * P, :]
                            .bitcast(F32R))
                        for d in range(nd):
                            for i, (o, w) in enumerate(hs_splits):
                                nc.tensor.matmul(
                                    phs[d][i][:, :], lhsT=mts[d][:, k, :],
                                    rhs=wch[:, o:o + w],
                                    start=(k == 0), stop=(k == nk - 1))
                    for d in range(nd):
                        nv = min(P, self.D1 - d * P)
                        for i, (o, w) in enumerate(hs_splits):
                            nc.vector.tensor_copy(out=h1[d][:, o:o + w],
                                                  in_=phs[d][i][:, :])
                        sq = hp.tile([P, hid], F32R, tag="b1sq",
                                     name="b1_sq")
                        nc.vector.tensor_tensor(out=sq[:nv, :],
                                                in0=h1[d][:nv, :],
                                                in1=h1[d][:nv, :], op=AOT.mult)
                        st, sp = (d == 0), (d == nd - 1)
                        for i, (o, w) in enumerate(hs_splits):
                            nc.tensor.matmul(ps_s[i][:, :],
                                             lhsT=ones_c[:nv, :],
                                             rhs=h1[d][:nv, o:o + w],
                                             start=st, stop=sp)
                            nc.tensor.matmul(ps_q[i][:, :],
                                             lhsT=ones_c[:nv, :],
                                             rhs=sq[:nv, o:o + w],
                                             start=st, stop=sp)
                for i, (o, w) in enumerate(hs_splits):
                    nc.vector.tensor_copy(out=stv[:, o:o + w],
                                          in_=ps_s[i][:, :])
                    nc.vector.tensor_copy(out=stv[:, hid + o:hid + o + w],
                                          in_=ps_q[i][:, :])
            nc.sync.dma_start(out=self.st_in1[:, :], in_=stv[:, :])
            nc.gpsimd.collective_compute(
                "AllReduce", AOT.add, replica_groups=self.rg,
                ins=[self.st_in1[:, :]], outs=[self.st_out1[:, :]])
            # scope 2: weights/means pools closed; BN + transpose + MLP head
            with tc.tile_pool(name="b1_sg", bufs=1) as sgp, \
                 tc.tile_pool(name="b1_t", bufs=2) as tp:
                sts = sgp.tile([1, 2 * hid], F32, name="b1_sts")
                nc.sync.dma_start(out=sts[:, :], in_=self.st_out1[:, :])
                scale, shift = self._bn_vec(tc, sgp, sts, self.g1, self.be1,
                                            c["N_DST1"], hid, "bn1")
                with tc.tile_pool(name="b1_psb", bufs=2, space="PSUM") as psb:
                    scB, shB = self._bn_bcast(tc, sgp, psb, scale, shift,
                                              hid, "bn1")
                for d in range(nd):
                    nv = min(P, self.D1 - d * P)
                    self._bn_elu_tile(tp, h1[d], nv, scB, shB, f"c1_{d}")

                # transpose h1 -> h1T [hid rows, D1 cols]
                ident = sgp.tile([P, P], F32, name="b1_id")
                from concourse.masks import make_identity
                make_identity(nc, ident[:, :])
                h1T = sgp.tile([P, nch, nd * P], F32R, name="b1_h1T")
                with tc.tile_pool(name="b1_pst", bufs=3, space="PSUM") as pstp:
                    for d in range(nd):
                        for f in range(nch):
                            pst = pstp.tile([P, P], F32, tag="pst",
                                            name="b1_pst")
                            nc.tensor.transpose(
                                pst[:, :],
                                h1[d][:, f * P:(f + 1) * P].bitcast(F32),
                                ident[:, :])
                            nc.vector.tensor_copy(
                                out=h1T[:, f, d * P:(d + 1) * P],
                                in_=pst[:, :])

                # MLP layer 1: z_preT[n2, d] = mlp_w1.T-chunks @ h1T
                w1s = sgp.tile([P, nch, hid], F32R, name="b1_w1s")
                nc.sync.dma_start(
                    out=w1s[:, :, :],
                    in_=self.mlp_w1[:, :].rearrange("(k p) n -> p k n",
                                                    p=P).bitcast(F32R))
                zT = sgp.tile([P, nch, nd * P], F32R, name="b1_zT")
                with tc.tile_pool(name="b1_psz", bufs=2, space="PSUM") as psz:
                    for n2 in range(nch):
                        pz = psz.tile([P, nd * P], F32, tag="pz", name="b1_pz")
                        for k in range(nch):
                            nc.tensor.matmul(
                                pz[:, :],
                                lhsT=w1s[:, k, n2 * P:(n2 + 1) * P],
                                rhs=h1T[:, k, :],
                                start=(k == 0), stop=(k == nch - 1))
                        nc.vector.tensor_copy(out=zT[:, n2, :], in_=pz[:, :])

                # BN2 (transposed): per-partition stats over free dim
                st2 = sgp.tile([P, 2 * nch], F32, name="b1_st2")
                for n2 in range(nch):
                    nc.vector.tensor_reduce(out=st2[:, n2:n2 + 1],
                                            in_=zT[:, n2, :],
                                            axis=mybir.AxisListType.X,
                                            op=AOT.add)
                    sq2 = tp.tile([P, nd * P], F32, tag="sq2", name="b1_sq2")
                    nc.vector.tensor_tensor(out=sq2[:, :], in0=zT[:, n2, :],
                                            in1=zT[:, n2, :], op=AOT.mult)
                    nc.vector.tensor_reduce(out=st2[:, nch + n2:nch + n2 + 1],
                                            in_=sq2[:, :],
                                            axis=mybir.AxisListType.X,
                                            op=AOT.add)
                nc.sync.dma_start(out=self.st_in2[:, :], in_=st2[:, :])
                nc.gpsimd.collective_compute(
                    "AllReduce", AOT.add, replica_groups=self.rg,
                    ins=[self.st_in2[:, :]], outs=[self.st_out2[:, :]])
                st2r = sgp.tile([P, 2 * nch], F32, name="b1_st2r")
                nc.sync.dma_start(out=st2r[:, :], in_=self.st_out2[:, :])
                # per-chunk scale/shift [P, 1] and fused apply + relu
                gT = sgp.tile([P, nch], F32, name="b1_gT")
                nc.sync.dma_start(out=gT[:, :], in_=bass.AP(
                    tensor=self.mlp_g, offset=0, ap=[[1, P], [P, nch]]))
                bT = sgp.tile([P, nch], F32, name="b1_bT")
                nc.sync.dma_start(out=bT[:, :], in_=bass.AP(
                    tensor=self.mlp_be, offset=0, ap=[[1, P], [P, nch]]))
                n1 = float(c["N_DST1"])
                mu2 = sgp.tile([P, nch], F32, name="b1_mu2")
                nc.vector.tensor_scalar(out=mu2[:, :], in0=st2r[:, :nch],
                                        scalar1=1.0 / n1, scalar2=None,
                                        op0=AOT.mult)
                m22 = sgp.tile([P, nch], F32, name="b1_m22")
                nc.vector.tensor_scalar(out=m22[:, :], in0=st2r[:, nch:],
                                        scalar1=1.0 / n1, scalar2=None,
                                        op0=AOT.mult)
                var2 = sgp.tile([P, nch], F32, name="b1_var2")
                nc.vector.tensor_tensor(out=var2[:, :], in0=mu2[:, :],
                                        in1=mu2[:, :], op=AOT.mult)
                nc.vector.tensor_tensor(out=var2[:, :], in0=m22[:, :],
                                        in1=var2[:, :], op=AOT.subtract)
                eps2 = sgp.tile([P, 1], F32, name="b1_eps2")
                nc.vector.memset(eps2[:, :], EPS)
                sd2 = sgp.tile([P, nch], F32, name="b1_sd2")
                nc.scalar.activation(out=sd2[:, :], in_=var2[:, :],
                                     func=ACT.Sqrt, bias=eps2[:, :])
                rstd2 = sgp.tile([P, nch], F32, name="b1_rstd2")
                nc.vector.reciprocal(out=rstd2[:, :], in_=sd2[:, :])
                sc2 = sgp.tile([P, nch], F32, name="b1_sc2")
                nc.vector.tensor_tensor(out=sc2[:, :], in0=gT[:, :],
                                        in1=rstd2[:, :], op=AOT.mult)
                sh2 = sgp.tile([P, nch], F32, name="b1_sh2")
                nc.vector.tensor_tensor(out=sh2[:, :], in0=mu2[:, :],
                                        in1=sc2[:, :], op=AOT.mult)
                nc.vector.tensor_tensor(out=sh2[:, :], in0=bT[:, :],
                                        in1=sh2[:, :], op=AOT.subtract)
                for n2 in range(nch):
                    nc.vector.tensor_scalar(out=zT[:, n2, :],
                                            in0=zT[:, n2, :],
                                            scalar1=sc2[:, n2:n2 + 1],
                                            scalar2=sh2[:, n2:n2 + 1],
                                            op0=AOT.mult, op1=AOT.add)
                    nc.vector.tensor_scalar(out=zT[:, n2, :],
                                            in0=zT[:, n2, :],
                                            scalar1=0.0, scalar2=None,
                                            op0=AOT.max)

                # final: out = zT.T-chunks @ mlp_w2 + b2
                oc = c["OUT_C"]
                ocp = self.OCP
                w2s = sgp.tile([P, nch, ocp], F32R, name="b1_w2s")
                nc.sync.dma_start(
                    out=w2s[:, :, :],
                    in_=self.mlp_w2[:, :].rearrange("(k p) n -> p k n",
                                                    p=P).bitcast(F32R))
                b2B = sgp.tile([P, oc], F32, name="b1_b2B")
                nc.gpsimd.dma_start(out=b2B[:, :], in_=bass.AP(
                    tensor=self.mlp_b2, offset=0, ap=[[0, P], [1, oc]]))
                with tc.tile_pool(name="b1_pso", bufs=2, space="PSUM") as pso:
                    for dh in range((self.D1 + P - 1) // P):
                        nv = min(P, self.D1 - dh * P)
                        po = pso.tile([P, ocp], F32, tag="po", name="b1_po")
                        for k in range(nch):
                            nc.tensor.matmul(
                                po[:nv, :],
                                lhsT=zT[:, k, dh * P:dh * P + nv],
                                rhs=w2s[:, k, :],
                                start=(k == 0), stop=(k == nch - 1))
                        ot = tp.tile([P, oc], F32, tag="ot", name="b1_ot")
                        nc.vector.tensor_tensor(out=ot[:nv, :],
                                                in0=po[:nv, :oc],
                                                in1=b2B[:nv, :], op=AOT.add)
                        nc.sync.dma_start(
                            out=self.out[dh * P:dh * P + nv, :],
                            in_=ot[:nv, :])

    def build(self):
        nc = self.nc
        with tile.TileContext(nc) as tc:
            self.phase_A(tc, "a0", self.x_tab, self.src0, self.col0, self.w0,
                         self.groups0, self.T0, self.cfg["IN_C"], self.meansT0)
            self.phase_B0(tc)
            self.phase_A(tc, "a1", self.h0_full, self.src1, self.col1,
                         self.w1, self.groups1, self.T1, self.cfg["HID"],
                         self.meansT1)
            self.phase_B1(tc)
        nc.compile()
        return nc


# ---------------------------------------------------------------------------
# top-level entry
# ---------------------------------------------------------------------------

def build_all(inputs, cfg):
    inputs = {k: np.asarray(v) for k, v in inputs.items()}
    g0, pc0, D0, NB0 = prep_layer(inputs["src0"], inputs["dst0"],
                                  inputs["et0"], cfg["N_DST0"], cfg["N_ET"],
                                  NCORES)
    g1, pc1, D1, NB1 = prep_layer(inputs["src1"], inputs["dst1"],
                                  inputs["et1"], cfg["N_DST1"], cfg["N_ET"],
                                  NCORES)
    T0 = sum(g[2] for g in g0)
    T1 = sum(g[2] for g in g1)
    W0, W1 = prep_weights(inputs, cfg["N_ET"])
    prog = Prog(cfg, g0, T0, g1, T1)
    nc = prog.build()

    shared = dict(
        x_tab=np.ascontiguousarray(inputs["x"], np.float32),
        Wstack0=W0, Wstack1=W1,
        mlp_w1=np.ascontiguousarray(inputs["mlp_w1"], np.float32),
        mlp_w2=np.ascontiguousarray(np.pad(
            np.asarray(inputs["mlp_w2"], np.float32),
            ((0, 0), (0, np.asarray(inputs["mlp_w2"]).shape[1] % 2))), np.float32),
        mlp_b2=np.ascontiguousarray(inputs["mlp_b2"], np.float32).reshape(1, -1),
        g0v=np.ascontiguousarray(inputs["g0"], np.float32).reshape(1, -1),
        be0v=np.ascontiguousarray(inputs["be0"], np.float32).reshape(1, -1),
        g1v=np.ascontiguousarray(inputs["g1"], np.float32).reshape(1, -1),
        be1v=np.ascontiguousarray(inputs["be1"], np.float32).reshape(1, -1),
        mlp_gv=np.ascontiguousarray(inputs["mlp_g"], np.float32).reshape(1, -1),
        mlp_bev=np.ascontiguousarray(inputs["mlp_be"], np.float32).reshape(1, -1),
    )
    in_maps = []
    for c in range(NCORES):
        m = dict(shared)
        m["src0_t"] = pc0[c]["src"]
        m["col0_t"] = pc0[c]["col"]
        m["w0_t"] = pc0[c]["w"]
        m["src1_t"] = pc1[c]["src"]
        m["col1_t"] = pc1[c]["col"]
        m["w1_t"] = pc1[c]["w"]
        in_maps.append(m)
    return nc, in_maps


LAST_RESULT = None


def kernel(**inputs) -> np.ndarray:
    global LAST_RESULT
    from concourse.bass_utils import run_bass_kernel_spmd
    cfg = dict(FULL_CFG)
    nc, in_maps = build_all(inputs, cfg)
    res = run_bass_kernel_spmd(nc, in_maps, core_ids=list(range(NCORES)))
    LAST_RESULT = res
    out = np.concatenate([res.results[c]["out_shard"] for c in range(NCORES)],
                         axis=0)
    return out.astype(np.float32)

